# revision 19
# baseline (speedup 1.0000x reference)
"""Sharded cosine-similarity kNN (k=5) + weighted centroid on 8 TRN2 NeuronCores.

Strategy (standard sharded kNN):
  - Shard the 500000x768 f32 database row-wise across 8 cores (62500 rows each,
    padded to 62592 rows = 489 column-groups of [128 rows x 768], padding with
    copies of -query, whose cosine similarity is exactly -1 and can never
    enter the top-k).
  - Each core streams its ~192 MB shard from HBM once. DMA-only sweeps show
    the stream rate is chunk-size sensitive: [128, 8*768] transfers (3.15 MB)
    reach ~349 GB/s/core (vs ~327 at [128, 4*768]); larger chunks gain
    nothing. So the main stream uses G=8 chunks, with two G=4 chunks at the
    end (smaller final tile => less non-overlapped compute after the last
    DMA) and the 1-column padded-tail tile emitted first.
  - Per 768-column row-group: DVE scalar_tensor_tensor (bypass,mult,
    accum_out) computes dot(row, q); ACT activation(Square, accum_out)
    computes ||row||^2. Every 16th square runs on DVE instead to balance
    engine busy (~7.6 us/tile vs 9.0 us DMA per G=8 tile).
  - Epilogue (sims = dots / max(sqrt(n2), eps)) is emitted in 64-column
    chunks interleaved with the stream so only ~17 columns + top-8
    max/max_index + an 8-KB output DMA remain after the last tile.
  - Host: gather 8x128x8 candidates, divide by ||q|| (order-preserving),
    global top-5, inverse-square-distance weights, tiny centroid gather.

Environment workaround: this container's walrus build rejects any instruction
with more than one semaphore wait; see split_sync_waits() below.
"""

import contextlib

import ml_dtypes
import numpy as np

import concourse.bass as bass
import concourse.mybir as mybir
from concourse.tile import TileContext
from concourse.bass_utils import run_bass_kernel_spmd

N_CORES = 8
D = 768
N_ROWS = 500000
SHARD = N_ROWS // N_CORES   # 62500
P = 128
NCOLS = 489                 # 62592 padded rows / 128
PAD_ROWS = NCOLS * P        # 62592 (92 pad rows)
GMAX = 8
K = 5
COS_EPS = 1e-8
W_EPS = 1e-6

# (col0, g) chunks in DMA-emission order: the padded-tail column first (keeps
# it off the critical end), then 60 G=8 chunks, then two G=4 chunks so the
# last tile's compute shadow is short. Column c covers shard rows
# col0*128 + p*g + (c - col0) for partition p.
CHUNK_PLAN = (
    [(488, 1)]
    + [(i * 8, 8) for i in range(60)]
    + [(480, 4), (484, 2), (486, 2)]
)
assert sorted(c for c0, g in CHUNK_PLAN for c in range(c0, c0 + g)) == list(
    range(NCOLS)
)

_f32 = mybir.dt.float32
_bf16 = mybir.dt.bfloat16
_u32 = mybir.dt.uint32

_wsplit_ctr = [0]


def split_sync_waits(nc):
    """Workaround for this container's walrus build: it rejects any instruction
    carrying more than ONE semaphore wait ("Too many sync wait commands" in
    setupSyncWait during codegen). Tile's scheduler freely attaches several
    waits to one instruction, so after TileContext scheduling we split them:
    every instruction keeps its last wait, and each extra wait is hoisted onto
    its own NoOp placed immediately before it in the same basic block (same
    engine, so program order preserves wait-before-execute semantics)."""
    for f in nc.m.functions:
        for b in f.blocks:
            needs_fix = any(
                getattr(i, "sync_info", None) is not None
                and i.sync_info.on_wait
                and len(i.sync_info.on_wait) > 1
                for i in b.instructions
            )
            if not needs_fix:
                continue
            new_insts = []
            for inst in b.instructions:
                si = getattr(inst, "sync_info", None)
                if si is not None and si.on_wait and len(si.on_wait) > 1:
                    waits = list(si.on_wait)
                    for w in waits[:-1]:
                        _wsplit_ctr[0] += 1
                        nop = mybir.InstNoOp(
                            name=f"WSPLIT-{_wsplit_ctr[0]}", ins=[], outs=[]
                        )
                        nop.engine = inst.engine
                        nop.sync_info = mybir.SyncInfo(on_wait=[w], on_update=[])
                        new_insts.append(nop)
                    inst.sync_info = mybir.SyncInfo(
                        on_wait=[waits[-1]], on_update=list(si.on_update or [])
                    )
                new_insts.append(inst)
            b.instructions[:] = new_insts
    return nc


def build_nc(db_bufs: int = 6, repeat: int = 1, dve_sq_every: int = 16,
             epi_every: int = 64, out_space: str = "SBUF",
             rings: tuple = ("sync",), chunk_plan: list | None = None,
             aux_ring: str = "scalar"):
    """repeat>1 wraps the body in tc.For_i for on-device timing (one NEFF).
    dve_sq_every=k moves every k-th square op from ACT to DVE (0 = all ACT).
    epi_every: emit the sims epilogue for each completed chunk of this many
    columns, interleaved with the DMA stream. out_space: where the unused
    elementwise outputs of the accumulating ops live ("PSUM" keeps their
    write streams off the SBUF ports the DMA stream needs). rings: HWDGE
    issue engines cycled per chunk ("sync"/"scalar")."""
    if chunk_plan is None:
        chunk_plan = CHUNK_PLAN
    gmax = max(g for _, g in chunk_plan)
    nc = bass.Bass()
    db = nc.dram_tensor("db", [PAD_ROWS * D], _bf16, kind="ExternalInput")
    qrep = nc.dram_tensor("qrep", [P, D], _bf16, kind="ExternalInput")
    outv = nc.dram_tensor("outv", [P, 8], _f32, kind="ExternalOutput")
    outi = nc.dram_tensor("outi", [P, 8], _u32, kind="ExternalOutput")

    ew_bufs = 3 if out_space == "SBUF" else 2
    with TileContext(nc) as tc:
        with (
            tc.tile_pool(name="persist", bufs=1) as persist,
            tc.tile_pool(name="dbp", bufs=db_bufs) as dbp,
            tc.tile_pool(name="dv", bufs=ew_bufs, space=out_space) as dvp,
            tc.tile_pool(name="da", bufs=ew_bufs, space=out_space) as dap,
        ):
            # qt and the tiny result DMAs ride the aux (scalar-engine HWDGE)
            # ring: their waits depend on compute, and on the sync ring they
            # would stall the in-order db stream (visible as a ~10us/iter
            # bubble in the repeat-loop proxy).
            aux = getattr(nc, aux_ring)
            loop = tc.For_i(0, repeat, 1) if repeat > 1 else contextlib.nullcontext()
            with loop:
                qt = persist.tile([P, D], _bf16, tag="qt")
                aux.dma_start(qt[:], qrep[:])

                dots = persist.tile([P, NCOLS], _f32, tag="dots")
                n2 = persist.tile([P, NCOLS], _f32, tag="n2")
                dn = persist.tile([P, NCOLS], _f32, tag="dn")
                inv = persist.tile([P, NCOLS], _f32, tag="inv")
                sims = persist.tile([P, NCOLS], _f32, tag="sims")

                nproc = [0]

                def process(sb_ap, col):
                    tout = dvp.tile([P, D], _bf16, tag="tout")
                    nc.vector.scalar_tensor_tensor(
                        out=tout[:],
                        in0=sb_ap,
                        scalar=0.0,
                        in1=qt[:],
                        op0=mybir.AluOpType.bypass,
                        op1=mybir.AluOpType.mult,
                        accum_out=dots[:, col : col + 1],
                    )
                    nproc[0] += 1
                    # ACT square+accum (~1.0us) vs DVE dot (~0.9us): shifting
                    # every 16th square to DVE balances both engines.
                    if dve_sq_every and nproc[0] % dve_sq_every == 0:
                        sq = dvp.tile([P, D], _bf16, tag="tout")
                        nc.vector.scalar_tensor_tensor(
                            out=sq[:],
                            in0=sb_ap,
                            scalar=0.0,
                            in1=sb_ap,
                            op0=mybir.AluOpType.bypass,
                            op1=mybir.AluOpType.mult,
                            accum_out=n2[:, col : col + 1],
                        )
                    else:
                        aout = dap.tile([P, D], _bf16, tag="aout")
                        nc.scalar.activation(
                            out=aout[:],
                            in_=sb_ap,
                            func=mybir.ActivationFunctionType.Square,
                            accum_out=n2[:, col : col + 1],
                        )

                def epilogue_chunk(lo, hi):
                    if hi <= lo:
                        return
                    nc.scalar.sqrt(dn[:, lo:hi], n2[:, lo:hi])
                    nc.vector.tensor_scalar_max(dn[:, lo:hi], dn[:, lo:hi], COS_EPS)
                    nc.vector.reciprocal(inv[:, lo:hi], dn[:, lo:hi])
                    nc.vector.tensor_mul(sims[:, lo:hi], dots[:, lo:hi],
                                         inv[:, lo:hi])

                epi_done = [0]

                def maybe_epilogue(complete_cols):
                    # columns [0, complete_cols) are fully accumulated
                    hi = (complete_cols // epi_every) * epi_every
                    if hi > epi_done[0]:
                        epilogue_chunk(epi_done[0], hi)
                        epi_done[0] = hi

                for ti, (col0, g) in enumerate(chunk_plan):
                    free = g * D
                    sb = dbp.tile([P, gmax * D], _bf16, tag="sb")
                    src = db[col0 * P * D : (col0 + g) * P * D]
                    eng = getattr(nc, rings[ti % len(rings)])
                    eng.dma_start(
                        sb[:, :free], src.rearrange("(p f) -> p f", f=free)
                    )
                    for j in range(g):
                        process(sb[:, j * D : (j + 1) * D], col0 + j)
                    if col0 != 488:
                        maybe_epilogue(col0 + g)

                epilogue_chunk(epi_done[0], NCOLS)

                vals8 = persist.tile([P, 8], _f32, tag="vals8")
                idx8 = persist.tile([P, 8], _u32, tag="idx8")
                nc.vector.max(vals8[:], sims[:])
                aux.dma_start(outv[:], vals8[:])
                nc.vector.max_index(idx8[:], vals8[:], sims[:])
                aux.dma_start(outi[:], idx8[:])
    split_sync_waits(nc)
    return nc


def _prep_inputs(query: np.ndarray, database: np.ndarray, n_cores: int = N_CORES,
                 shard: int = SHARD):
    """Build per-core input maps, quantized to bf16 (halves the HBM stream;
    exactness is restored host-side by re-scoring the top candidates against
    the original f32 rows). Pads each shard with copies of -query (cosine
    similarity ~-1: never selected)."""
    q = np.ascontiguousarray(np.asarray(query, dtype=np.float32)).reshape(1, D)
    db = np.asarray(database, dtype=np.float32)
    qrep = np.ascontiguousarray(
        np.tile(q, (P, 1)).astype(ml_dtypes.bfloat16)
    )  # [128, 768]
    in_maps = []
    for c in range(n_cores):
        sh = np.empty((PAD_ROWS, D), dtype=np.float32)
        sh[:shard] = db[c * shard : (c + 1) * shard]
        sh[shard:] = -q
        in_maps.append(
            {"db": sh.reshape(-1).astype(ml_dtypes.bfloat16), "qrep": qrep}
        )
    return in_maps


# per-column chunk lookup derived from CHUNK_PLAN
_COL0 = np.zeros(NCOLS, dtype=np.int64)
_CG = np.zeros(NCOLS, dtype=np.int64)
for _c0, _g in CHUNK_PLAN:
    _COL0[_c0 : _c0 + _g] = _c0
    _CG[_c0 : _c0 + _g] = _g


def _cols_to_rows(cols: np.ndarray, p_idx: np.ndarray) -> np.ndarray:
    """Map candidate column index (per partition) back to shard row:
    column c in chunk (col0, g) => row col0*128 + p*g + (c-col0)."""
    c0 = _COL0[cols]
    g = _CG[cols]
    return c0 * P + p_idx * g + (cols - c0)


def _host_reduce(results, query: np.ndarray, database: np.ndarray,
                 n_cores: int = N_CORES, shard: int = SHARD) -> np.ndarray:
    q = np.asarray(query, dtype=np.float32).reshape(1, D)
    db = np.asarray(database, dtype=np.float32)

    vals = np.stack([r["outv"] for r in results])          # [C,128,8] dot/||row||
    cols = np.stack([r["outi"] for r in results]).astype(np.int64)  # [C,128,8]

    c_idx = np.arange(n_cores, dtype=np.int64)[:, None, None]
    p_idx = np.arange(P, dtype=np.int64)[None, :, None]
    shard_row = _cols_to_rows(cols, p_idx)
    gidx = c_idx * shard + shard_row

    valid = (shard_row < shard).ravel()
    v = vals.ravel()[valid]
    g = gidx.ravel()[valid]

    # Device sims are bf16-precision (~5e-4): take a generous candidate pool
    # by device score, then re-score those rows exactly in f32/f64 so the
    # final top-5 and weights match the f32 reference bit-for-bit in practice.
    npool = min(64, v.size)
    pool = np.argpartition(-v, npool - 1)[:npool]
    cand = g[pool]

    qn = max(float(np.linalg.norm(q.astype(np.float64))), COS_EPS)
    rows = db[cand].astype(np.float64)
    dn_c = np.maximum(np.linalg.norm(rows, axis=1), COS_EPS)
    sims_c = (rows @ q.astype(np.float64)[0]) / (dn_c * qn)

    top = np.argsort(-sims_c.astype(np.float32), kind="stable")[:K]
    s = sims_c[top]
    idx = cand[top]

    d = 1.0 - s
    w = 1.0 / (d + W_EPS) ** 2
    w = w / w.sum()
    centroid = (w[None, :] @ db[idx].astype(np.float64)).astype(np.float32)
    return centroid  # [1, D]


def _run(query: np.ndarray, database: np.ndarray, trace: bool = False):
    nc = build_nc()
    in_maps = _prep_inputs(query, database)
    res = run_bass_kernel_spmd(
        nc, in_maps, core_ids=list(range(N_CORES)), trace=trace,
    )
    out = _host_reduce(res.results, query, database)
    return out, res


def kernel(query: np.ndarray, database: np.ndarray) -> np.ndarray:
    out, _ = _run(query, database, trace=False)
    return out
